# revision 3
# baseline (speedup 1.0000x reference)
"""Bilateral-grid slice kernel v2 for Trainium2 (8 NeuronCores, SPMD).

Algorithm changes vs v1:
  - epsilon tables: device works on (grid - identity); out = rgb + interp(eps)@u.
    All weight/table noise couples only to |eps|~0.05, so bf16 everywhere.
  - contraction over (y,x) = 256 (two 128-slabs, PSUM-accumulated bf16 matmuls,
    rhs = eps-table [128, 96]); tail is the z-interp (12ch x 8z) -> much less
    DVE work than the x-tail.
  - one-hot transposes via DMA xbar transpose (bf16, SBUF->SBUF), freeing both
    the PE (no transpose matmuls) and ACT (no PSUM->SBUF copies).
  - tail reduce as a bf16 tensor-tensor add tree (2x mode) instead of
    tensor_reduce (which has no fast mode).
"""

import numpy as np
from contextlib import ExitStack

import concourse.bacc as bacc
import concourse.bass as bass
import concourse.tile as tile
import concourse.mybir as mybir
from concourse.bass_utils import run_bass_kernel_spmd

F32 = mybir.dt.float32
BF16 = mybir.dt.bfloat16
ALU = mybir.AluOpType
ACTFN = mybir.ActivationFunctionType

NVIEW, L, GH, GW = 2, 8, 16, 16
IMG_H, IMG_W = 1080, 1920
NCORES = 8
P = 128

ROWS_PER_CORE = IMG_H // 4                      # 270
PIX_PER_CORE = ROWS_PER_CORE * IMG_W            # 518400
CTOT = PIX_PER_CORE // P                        # 4050
CHUNK = 270                                     # px-cols per chunk
NCHUNK = CTOT // CHUNK                          # 30
JG = 15                                         # px-cols per j-group
NGRP = CHUNK // JG                              # 9

GRAY_R, GRAY_G, GRAY_B = 0.299, 0.587, 0.114


def _ap(base: bass.AP, offset_add: int, free_dims):
    return bass.AP(base.tensor, base.offset + offset_add, [base.ap[0]] + free_dims)


def build_module():
    nc = bacc.Bacc("TRN2", target_bir_lowering=False, debug=False,
                   num_devices=NCORES)

    inp_all = nc.dram_tensor("inp_all", [P, NCHUNK * 5 * CHUNK], F32,
                             kind="ExternalInput").ap()
    gbd = nc.dram_tensor("gb", [P, 192], F32, kind="ExternalInput").ap()
    cst = nc.dram_tensor("cst", [1, 40], F32, kind="ExternalInput").ap()
    out = nc.dram_tensor("out", [P, 3 * CTOT], F32, kind="ExternalOutput").ap()

    with tile.TileContext(nc) as tc:
        with ExitStack() as ctx:
            cpool = ctx.enter_context(tc.tile_pool(name="const", bufs=1))
            inp = ctx.enter_context(tc.tile_pool(name="inp", bufs=2))
            hatp = ctx.enter_context(tc.tile_pool(name="hat", bufs=2))
            hyep = ctx.enter_context(tc.tile_pool(name="hye", bufs=3))
            spool = ctx.enter_context(tc.tile_pool(name="sprod", bufs=3))
            stp = ctx.enter_context(tc.tile_pool(name="st", bufs=4))
            vp = ctx.enter_context(tc.tile_pool(name="v_ps", bufs=2, space="PSUM"))
            wp = ctx.enter_context(tc.tile_pool(name="w", bufs=3))
            tp = ctx.enter_context(tc.tile_pool(name="tree", bufs=3))
            dap = ctx.enter_context(tc.tile_pool(name="da", bufs=2))
            opool = ctx.enter_context(tc.tile_pool(name="outb", bufs=2))

            # constants
            gb_f = cpool.tile([P, 192], F32)
            nc.sync.dma_start(gb_f[:], gbd)
            gb_b = cpool.tile([P, 192], BF16)
            nc.scalar.copy(gb_b[:], gb_f[:])
            zc = cpool.tile([P, 8], F32)
            nc.sync.dma_start(zc[:], cst[0:1, 0:8].to_broadcast((P, 8)))
            yc = cpool.tile([P, 16], F32)
            nc.sync.dma_start(yc[:], cst[0:1, 8:24].to_broadcast((P, 16)))
            xc = cpool.tile([P, 16], F32)
            nc.sync.dma_start(xc[:], cst[0:1, 24:40].to_broadcast((P, 16)))

            it_next = inp.tile([P, 5 * CHUNK], F32, tag="it")
            nc.sync.dma_start(it_next[:], inp_all[:, 0:5 * CHUNK])
            for ci in range(NCHUNK):
                cb = ci * CHUNK
                it = it_next
                if ci + 1 < NCHUNK:
                    it_next = inp.tile([P, 5 * CHUNK], F32, tag="it")
                    nc.sync.dma_start(
                        it_next[:],
                        inp_all[:, (ci + 1) * 5 * CHUNK:(ci + 2) * 5 * CHUNK])
                xt, yt = it[:, 0:CHUNK], it[:, CHUNK:2 * CHUNK]
                rt = it[:, 2 * CHUNK:3 * CHUNK]
                gt = it[:, 3 * CHUNK:4 * CHUNK]
                bt = it[:, 4 * CHUNK:5 * CHUNK]

                # gray precursor: gray = GRAY_G * t2
                t1 = inp.tile([P, CHUNK], F32, tag="t1")
                nc.vector.scalar_tensor_tensor(
                    t1[:], rt, GRAY_R / GRAY_G, gt,
                    op0=ALU.mult, op1=ALU.add)
                t2 = inp.tile([P, CHUNK], F32, tag="t2")
                nc.vector.scalar_tensor_tensor(
                    t2[:], bt, GRAY_B / GRAY_G, t1[:],
                    op0=ALU.mult, op1=ALU.add)

                # hat args -> one bf16 tile, per-j layout [z8 | y16 | x16]
                hat = hatp.tile([P, 40 * CHUNK], BF16, tag="hat")
                nc.vector.scalar_tensor_tensor(
                    _ap(hat[:], 0, [[40, CHUNK], [1, 8]]),
                    _ap(t2[:], 0, [[1, CHUNK], [0, 8]]),
                    GRAY_G * (L - 1),
                    _ap(zc[:], 0, [[0, CHUNK], [1, 8]]),
                    op0=ALU.mult, op1=ALU.subtract)
                nc.vector.scalar_tensor_tensor(
                    _ap(hat[:], 8, [[40, CHUNK], [1, 16]]),
                    _ap(yt, 0, [[1, CHUNK], [0, 16]]),
                    float(GH - 1),
                    _ap(yc[:], 0, [[0, CHUNK], [1, 16]]),
                    op0=ALU.mult, op1=ALU.subtract)
                nc.vector.scalar_tensor_tensor(
                    _ap(hat[:], 24, [[40, CHUNK], [1, 16]]),
                    _ap(xt, 0, [[1, CHUNK], [0, 16]]),
                    float(GW - 1),
                    _ap(xc[:], 0, [[0, CHUNK], [1, 16]]),
                    op0=ALU.mult, op1=ALU.subtract)

                # hat(t) = relu(1 - |t|)
                nc.scalar.activation(hat[:], hat[:], ACTFN.Abs)
                nc.scalar.activation(hat[:], hat[:], ACTFN.Relu,
                                     bias=1.0, scale=-1.0)

                dA = dap.tile([P, 12 * CHUNK], F32, tag="dA")

                for g in range(NGRP):
                    jb = g * JG
                    hb = jb * 40
                    # hyE[j, y, x] = hy[j, y] (ACT expand-copy; the stride-0
                    # -last bf16 read is only safe on ACT)
                    hyE = hyep.tile([P, 256 * JG], BF16, tag="hyE")
                    nc.scalar.copy(
                        hyE[:].rearrange("p (j y x) -> p j y x", y=16, x=16),
                        _ap(hat[:], hb + 8, [[40, JG], [1, 16], [0, 16]]))
                    sg = spool.tile([P, 256 * JG], BF16, tag="sg")
                    nc.vector.tensor_tensor(
                        sg[:].rearrange("p (j y x) -> p j y x", y=16, x=16),
                        hyE[:].rearrange("p (j y x) -> p j y x", y=16, x=16),
                        _ap(hat[:], hb + 24, [[40, JG], [0, 16], [1, 16]]),
                        op=ALU.mult)
                    # DMA xbar block-transpose: st[p, b, q] = sg[q, b*128+p]
                    # alternate issue between the two HWDGE queues (SP / ACT)
                    st = stp.tile([P, 256 * JG], BF16, tag="st")
                    nc.sync.dma_start(
                        st[:].rearrange("p (b q) -> p b q", q=P),
                        sg[:], transpose=True)

                    # V[j, ch, z] += S^T slabs @ eps tables
                    vt = vp.tile([P, 128 * JG], F32)
                    for j in range(JG):
                        nc.tensor.matmul(
                            _ap(vt[:], j * 128, [[1, 96]]),
                            lhsT=_ap(st[:], (2 * j) * P, [[1, P]]),
                            rhs=gb_b[:, 0:96], start=True, stop=False)
                        nc.tensor.matmul(
                            _ap(vt[:], j * 128, [[1, 96]]),
                            lhsT=_ap(st[:], (2 * j + 1) * P, [[1, P]]),
                            rhs=gb_b[:, 96:192], start=False, stop=True)

                    # w[j, ch, z] = V * hz   (f32 psum read, bf16 out)
                    w = wp.tile([P, 96 * JG], BF16, tag="w")
                    nc.vector.tensor_tensor(
                        w[:].rearrange("p (j c z) -> p j c z", c=12, z=8),
                        _ap(vt[:], 0, [[128, JG], [8, 12], [1, 8]]),
                        _ap(hat[:], hb, [[40, JG], [0, 12], [1, 8]]),
                        op=ALU.mult)
                    # z-reduce tree (bf16 2x adds)
                    t1t = tp.tile([P, 48 * JG], BF16, tag="t1t")
                    nc.vector.tensor_tensor(
                        t1t[:].rearrange("p (j c z) -> p j c z", c=12, z=4),
                        _ap(w[:], 0, [[96, JG], [8, 12], [1, 4]]),
                        _ap(w[:], 4, [[96, JG], [8, 12], [1, 4]]),
                        op=ALU.add)
                    t2t = tp.tile([P, 24 * JG], BF16, tag="t2t")
                    nc.vector.tensor_tensor(
                        t2t[:].rearrange("p (j c z) -> p j c z", c=12, z=2),
                        _ap(t1t[:], 0, [[48, JG], [4, 12], [1, 2]]),
                        _ap(t1t[:], 2, [[48, JG], [4, 12], [1, 2]]),
                        op=ALU.add)
                    nc.vector.tensor_tensor(
                        _ap(dA[:], jb * 12, [[12, JG], [1, 12]]),
                        _ap(t2t[:], 0, [[24, JG], [2, 12]]),
                        _ap(t2t[:], 1, [[24, JG], [2, 12]]),
                        op=ALU.add)

                # apply on GPSIMD: out_i = rgb_i + dA[4i+3] + sum_j dA[4i+j]*rgb_j
                ot = opool.tile([P, 3 * CHUNK], F32, tag="ot")
                rgbt = (rt, gt, bt)
                for i in range(3):
                    m = []
                    for j in range(3):
                        mj = opool.tile([P, CHUNK], F32, tag=f"m{j}")
                        nc.gpsimd.tensor_tensor(
                            mj[:],
                            _ap(dA[:], 4 * i + j, [[12, CHUNK]]),
                            rgbt[j], op=ALU.mult)
                        m.append(mj)
                    s2 = opool.tile([P, CHUNK], F32, tag="s2")
                    nc.gpsimd.tensor_tensor(
                        s2[:], _ap(dA[:], 4 * i + 3, [[12, CHUNK]]),
                        rgbt[i], op=ALU.add)
                    s01 = opool.tile([P, CHUNK], F32, tag="s01")
                    nc.gpsimd.tensor_tensor(s01[:], m[0][:], m[1][:], op=ALU.add)
                    s23 = opool.tile([P, CHUNK], F32, tag="s23")
                    nc.gpsimd.tensor_tensor(s23[:], m[2][:], s2[:], op=ALU.add)
                    nc.gpsimd.tensor_tensor(
                        _ap(ot[:], i, [[3, CHUNK]]), s01[:], s23[:], op=ALU.add)

                nc.gpsimd.dma_start(out[:, 3 * cb:3 * (cb + CHUNK)], ot[:])

    nc.compile()
    return nc


_NC_CACHE = {}


def _get_module():
    if "m" not in _NC_CACHE:
        _NC_CACHE["m"] = build_module()
    return _NC_CACHE["m"]


def _make_core_inputs(grids, coords, rgb):
    consts = np.concatenate([
        np.arange(8, dtype=np.float32),
        np.arange(16, dtype=np.float32),
        np.arange(16, dtype=np.float32),
    ]).reshape(1, 40)
    eye12 = np.eye(4, dtype=np.float32)[:3].reshape(12)
    in_maps = []
    for core in range(NCORES):
        v, q = divmod(core, 4)
        r0, r1 = ROWS_PER_CORE * q, ROWS_PER_CORE * (q + 1)
        blk = lambda a: np.ascontiguousarray(a.reshape(P, CTOT), np.float32)
        # eps[ch, z, y, x] = grids[v] - identity
        eps = (np.asarray(grids[v], np.float32)
               - eye12[:, None, None, None]).astype(np.float32)
        # gb[c, s*96 + ch*8 + z] = eps[ch, z, s*8 + c//16, c%16]
        gb = np.empty((P, 192), np.float32)
        for s in range(2):
            # eps -> [y_local, x, ch, z]
            t = eps[:, :, s * 8:(s + 1) * 8, :].transpose(2, 3, 0, 1)
            gb[:, s * 96:(s + 1) * 96] = t.reshape(P, 96)
        planes = [blk(coords[v, 0, r0:r1, :, 0]), blk(coords[v, 0, r0:r1, :, 1]),
                  blk(rgb[v, 0, r0:r1, :, 0]), blk(rgb[v, 0, r0:r1, :, 1]),
                  blk(rgb[v, 0, r0:r1, :, 2])]
        # inp_all[p, ci*675 + k*135 + j] = plane_k[p, ci*135 + j]
        stack = np.stack([pl.reshape(P, NCHUNK, CHUNK) for pl in planes],
                         axis=2)          # [P, NCHUNK, 5, CHUNK]
        in_maps.append({
            "inp_all": np.ascontiguousarray(
                stack.reshape(P, NCHUNK * 5 * CHUNK), np.float32),
            "gb": gb,
            "cst": consts,
        })
    return in_maps


def _run(grids, coords, rgb, trace=False):
    nc = _get_module()
    in_maps = _make_core_inputs(grids, coords, rgb)
    res = run_bass_kernel_spmd(nc, in_maps, core_ids=list(range(NCORES)),
                               trace=trace)
    full = np.empty((NVIEW, 1, IMG_H, IMG_W, 3), np.float32)
    for core in range(NCORES):
        v, q = divmod(core, 4)
        o = res.results[core]["out"]
        full[v, 0, ROWS_PER_CORE * q:ROWS_PER_CORE * (q + 1)] = (
            o.reshape(P, CTOT, 3).reshape(ROWS_PER_CORE, IMG_W, 3))
    return full, res


def kernel(grids, coords, rgb):
    full, _ = _run(np.asarray(grids), np.asarray(coords), np.asarray(rgb))
    return full
